# revision 16
# baseline (speedup 1.0000x reference)
"""Trainium2 Bass kernel for nn_AttentionModel (4-layer gated transformer).

Sharding: pure data-parallel over batch (B=16 -> 2 per core, 8 cores, no
collectives). Feature-major activations in bf16 (fp32 PSUM accumulate).

Perf structure:
- Weights host-packed to bf16 blobs; one DMA per layer stage (attn 2.6MB,
  ff 6.3MB), prefetched a stage ahead. All biases/LN params in one fp32
  [128, NP] blob -> single DMA.
- All activation functions from the exp_and_others table set (exp/tanh/
  identity/copy/square) except LN Sqrt: sigmoid(z) folded to
  (tanh(z/2)+1) * 0.5 with the 0.5s folded into weights host-side.
- Softmax: probs = exp(s)*exp(p); exp(pos_bias) precomputed on host (bf16
  multiplicative factor) so scores go PSUM -ACT-> exp -TT-> probs (bf16).
  Score scale 1/8 folded into Wq. Denominators via ones-augmented V column;
  reciprocal_approx_fast (~5x faster than vector.reciprocal).
- LayerNorm: partition sums via PE ones-matmuls, batched stats for both
  512-token chunks in one [2,512] tile, recip_approx for 1/sigma.
"""

import os
import sys

for _p in ("/opt/trn_rl_repo",):
    if os.path.isdir(_p) and _p not in sys.path:
        sys.path.insert(0, _p)

import numpy as np
import ml_dtypes

import concourse.bass as bass
import concourse.mybir as mybir
import concourse.tile as tile
from concourse import bacc
from concourse.bass_utils import run_bass_kernel_spmd

F32 = mybir.dt.float32
F32R = mybir.dt.float32r
BF = mybir.dt.bfloat16
NPBF = ml_dtypes.bfloat16
AF = mybir.ActivationFunctionType
OP = mybir.AluOpType

B, S, FC, FO = 16, 512, 24, 16
D, H, DK, FFD, L = 512, 8, 64, 2048, 4
MAXPOS = 512
EPS = 1e-6

NCORES = 8
BL = B // NCORES          # local batch = 2
R = BL * S                # local tokens = 1024
DT = D // 128             # feature tiles = 4
FT = FFD // 128           # ff tiles = 16
HDK = H * DK

# aw blob column bases (per layer, [128, 10240] bf16)
AW_Q, AW_K, AW_V, AW_O, AW_G = 0, 2048, 4096, 6144, 8192
AW_COLS = 10240
# fw blob column bases ([128, 24576] bf16)
FW_1, FW_G, FW_2 = 0, 8192, 16384
FW_COLS = 24576
# param blob columns (fp32 [128, NP])
PL = 68                   # per-layer stride
# per-layer: bq 0, bk 4, bo 8, bg 12, l1s 16, l1b 20, l2s 24, l2b 28,
#            bf1 32, bfg 48, bf2 64
HB = L * PL               # head base = 272
# head: cgm_b +0, other_b +4, fb1 +8, fl1s +10, fl1b +12, fb2 +14,
#       fl2s +15, fl2b +16, fw3 +17, fb3 +18 (row 0)
NP = HB + 19

_CACHE = {}


def _build():
    nc = bacc.Bacc("TRN2", target_bir_lowering=False, debug=False,
                   num_devices=NCORES)

    def par(name, shape, dt):
        return nc.declare_dram_parameter(name, list(shape), dt, isOutput=False)

    xin_d = par("xin", [FC, R], BF)
    xo_d = par("xo", [FO, BL], BF)
    cgmW_d = par("cgmW", [FC, D], BF)
    posE_d = par("posE", [128, 4 * 512], BF)
    aw_d = par("aw", [L, 128, AW_COLS], BF)
    fw_d = par("fw", [L, 128, FW_COLS], BF)
    pb_d = par("pb", [128, NP], F32)
    bvr_d = par("bvr", [L, HDK], F32R)
    hw1_d = par("hw1", [128, 8 * 256], BF)
    hw2_d = par("hw2", [128, 2 * 128 + 1], BF)
    ow_d = par("ow", [FO, D], BF)
    out_ext = nc.declare_dram_parameter("out", [1, BL], F32, isOutput=True)

    with tile.TileContext(nc) as tc:
        with (
            nc.allow_low_precision(reason="bf16 matmul/activation pipeline"),
            tc.tile_pool(name="P", bufs=1) as P,
            tc.tile_pool(name="Q", bufs=1, space="PSUM") as Q,
        ):
            MM = nc.tensor.matmul
            NLAYERS = int(os.environ.get("KLAYERS", L))
            KPROBE = os.environ.get("KPROBE", "")
            if KPROBE:
                dbg_ext = nc.declare_dram_parameter(
                    "dbg", [128, 1024], F32, isOutput=True)
                dbg_done = [False]

                def probe(name, ap):
                    if name != KPROBE or dbg_done[0]:
                        return
                    dbg_done[0] = True
                    pdim = ap.shape[0]
                    fdim = ap.free_size()
                    dt_ = P.tile([128, 1024], F32, tag="dbgt", bufs=1)
                    nc.vector.memset(dt_, 0.0)
                    nc.vector.tensor_copy(
                        dt_[0:pdim, 0:fdim], ap)
                    nc.sync.dma_start(out=dbg_ext[:, :], in_=dt_)
            else:
                def probe(name, ap):
                    pass

            # ---------------- constants ----------------
            ones_col = P.tile([128, 1], BF, tag="c_oc", bufs=1)
            nc.vector.memset(ones_col, 1.0)
            ones_row_f = P.tile([1, 128], F32, tag="c_orf", bufs=1)
            nc.vector.memset(ones_row_f, 1.0)
            ones_row_r = P.tile([1, 128], F32R, tag="c_orr", bufs=1)
            nc.vector.tensor_copy(ones_row_r, ones_row_f)
            eps2 = P.tile([2, 1], F32, tag="c_e", bufs=1)
            nc.vector.memset(eps2, EPS)

            # ---------------- persistent loads ----------------
            pb_sb = P.tile([128, NP], F32, tag="pb", bufs=1)
            nc.sync.dma_start(out=pb_sb, in_=pb_d[:, :])
            posE_sb = P.tile([128, 2048], BF, tag="posE", bufs=1)
            nc.sync.dma_start(out=posE_sb, in_=posE_d[:, :])
            xin_sb = P.tile([FC, R], BF, tag="xin", bufs=1)
            nc.sync.dma_start(out=xin_sb, in_=xin_d[:, :])
            cgmW_sb = P.tile([FC, D], BF, tag="cgmW", bufs=1)
            nc.sync.dma_start(out=cgmW_sb, in_=cgmW_d[:, :])
            bvr_sb = []
            for l in range(L):
                t = P.tile([1, HDK], F32R, tag="bvr", bufs=L,
                           name=f"bvr{l}")
                nc.sync.dma_start(out=t, in_=bvr_d[l].unsqueeze(0))
                bvr_sb.append(t)

            def col(c, n=1):
                return pb_sb[:, c:c + n]

            # layer weight pools
            def load_aw(l, chunked=False):
                t = P.tile([128, AW_COLS], BF, tag="aw",
                           bufs=(1 if KPROBE else 2),
                           name=f"aw{l}")
                if chunked:
                    for c0 in range(0, AW_COLS, 2048):
                        nc.sync.dma_start(out=t[:, c0:c0 + 2048],
                                          in_=aw_d[l][:, c0:c0 + 2048])
                else:
                    nc.sync.dma_start(out=t, in_=aw_d[l])
                return t

            def load_fw(l):
                t = P.tile([128, FW_COLS], BF, tag="fw", bufs=1,
                           name=f"fw{l}")
                nc.sync.dma_start(out=t, in_=fw_d[l])
                return t

            aw_sb = load_aw(0, chunked=True)
            xo_sb = P.tile([FO, BL], BF, tag="xo", bufs=1)
            nc.sync.dma_start(out=xo_sb, in_=xo_d[:, :])
            ow_sb = P.tile([FO, D], BF, tag="ow", bufs=1)
            nc.sync.dma_start(out=ow_sb, in_=ow_d[:, :])
            hw1_sb = P.tile([128, 2048], BF, tag="hw1", bufs=1)
            nc.sync.dma_start(out=hw1_sb, in_=hw1_d[:, :])
            hw2_sb = P.tile([128, 257], BF, tag="hw2", bufs=1)
            nc.sync.dma_start(out=hw2_sb, in_=hw2_d[:, :])

            # ------------- activation tile allocator -------------
            free_tags = ["bA", "bB", "bC", "bD", "bE", "bF"]

            def alloc_act():
                tag = free_tags.pop(0)
                tiles = [P.tile([128, R], BF, tag=tag, bufs=4,
                                name=f"{tag}_{nc.next_id()}")
                         for _ in range(DT)]
                return tiles, tag

            def free_act(tag):
                free_tags.append(tag)

            # persistent token-major V (ones-augmented)
            vv = []
            for rt in range(8):
                t = P.tile([128, H * (DK + 1)], BF, tag="vv", bufs=8,
                           name=f"vv{rt}")
                v3 = t.rearrange("p (h e) -> p h e", e=DK + 1)
                nc.vector.memset(v3[:, :, DK:DK + 1], 1.0)
                vv.append(t)

            # ---------------- input projection ----------------
            xT, xT_tag = alloc_act()
            for nt in range(DT):
                for rc in range(2):
                    ps = Q.tile([128, 512], F32, tag="B", bufs=2)
                    MM(ps, cgmW_sb[:, nt * 128:(nt + 1) * 128],
                       xin_sb[:, rc * 512:(rc + 1) * 512],
                       start=True, stop=True)
                    nc.scalar.activation(
                        out=xT[nt][:, rc * 512:(rc + 1) * 512], in_=ps,
                        func=AF.Identity, bias=col(HB + nt))

            # ---------------- helpers ----------------
            def proj_v(dst, wbase, bcols, src, act=None):
                """dst[nt] = act(src @ W + b), feature-major.

                act=None: vector tensor_scalar add-bias (PSUM->bf16)
                act=AF.*: scalar activation with bias
                """
                for nt in range(DT):
                    for rc in range(2):
                        ps = Q.tile([128, 512], F32, tag="B", bufs=2,
                                    name=f"pj_{nc.next_id()}")
                        for kt in range(DT):
                            MM(ps,
                               aw_sb[:, wbase + kt * 512 + nt * 128:
                                     wbase + kt * 512 + nt * 128 + 128],
                               src[kt][:, rc * 512:(rc + 1) * 512],
                               start=(kt == 0), stop=(kt == DT - 1))
                        o = dst[nt][:, rc * 512:(rc + 1) * 512]
                        if act is None:
                            nc.vector.tensor_scalar(
                                out=o, in0=ps, scalar1=col(bcols + nt),
                                scalar2=None, op0=OP.add)
                        else:
                            nc.scalar.activation(out=o, in_=ps, func=act,
                                                 bias=col(bcols + nt))

            def layernorm(res, cs, cb, dst):
                """dst = LN(res) over features (partitions)."""
                for rc in range(2):
                    sl = slice(rc * 512, (rc + 1) * 512)
                    s1p = Q.tile([1, 512], F32, tag="B", bufs=2,
                                 name=f"s1_{nc.next_id()}")
                    s2p = Q.tile([1, 512], F32, tag="C", bufs=2,
                                 name=f"s2_{nc.next_id()}")
                    for kt in range(DT):
                        MM(s1p, ones_col, res[kt][:, sl],
                           start=(kt == 0), stop=(kt == DT - 1))
                    for kt in range(DT):
                        sq = P.tile([128, 512], BF, tag="scr", bufs=5,
                                    name=f"sq_{nc.next_id()}")
                        nc.vector.tensor_mul(sq, res[kt][:, sl],
                                             res[kt][:, sl])
                        MM(s2p, ones_col, sq,
                           start=(kt == 0), stop=(kt == DT - 1))
                    mu = P.tile([1, 512], F32R, tag="ln_mu", bufs=3,
                                name=f"mu_{nc.next_id()}")
                    nc.vector.tensor_scalar(out=mu, in0=s1p,
                                            scalar1=1.0 / D,
                                            scalar2=None, op0=OP.mult)
                    m2 = P.tile([1, 512], F32, tag="ln_t", bufs=3,
                                name=f"m2_{nc.next_id()}")
                    nc.vector.tensor_scalar(out=m2, in0=s2p,
                                            scalar1=1.0 / D,
                                            scalar2=None, op0=OP.mult)
                    var = P.tile([1, 512], F32, tag="ln_t", bufs=3,
                                 name=f"va_{nc.next_id()}")
                    nc.vector.scalar_tensor_tensor(
                        var, mu, -1.0, mu, op0=OP.mult, op1=OP.mult)
                    nc.vector.tensor_add(var, var, m2)
                    sg = P.tile([1, 512], F32, tag="ln_t", bufs=3,
                                name=f"sg_{nc.next_id()}")
                    nc.scalar.activation(out=sg, in_=var, func=AF.Sqrt,
                                         bias=eps2[0:1, :])
                    rs = P.tile([1, 512], F32, tag="ln_mu", bufs=3,
                                name=f"rs_{nc.next_id()}")
                    nc.vector.reciprocal_approx_fast(out=rs, in_=sg)
                    rsr = P.tile([1, 512], F32R, tag="ln_mu", bufs=3,
                                 name=f"rsr_{nc.next_id()}")
                    nc.vector.tensor_copy(rsr, rs)
                    mub = Q.tile([128, 512], F32, tag="C", bufs=2,
                                 name=f"mb_{nc.next_id()}")
                    MM(mub, ones_row_r, mu, start=True, stop=True)
                    rsb = Q.tile([128, 512], F32, tag="B", bufs=2,
                                 name=f"rb_{nc.next_id()}")
                    MM(rsb, ones_row_r, rsr, start=True, stop=True)
                    mub_bf = P.tile([128, 512], BF, tag="scr", bufs=5,
                                    name=f"mbb_{nc.next_id()}")
                    nc.scalar.activation(out=mub_bf, in_=mub, func=AF.Copy)
                    rsb_bf = P.tile([128, 512], BF, tag="scr", bufs=5,
                                    name=f"rbb_{nc.next_id()}")
                    nc.scalar.activation(out=rsb_bf, in_=rsb, func=AF.Copy)
                    for kt in range(DT):
                        t1 = P.tile([128, 512], BF, tag="scr", bufs=5,
                                    name=f"t1_{nc.next_id()}")
                        nc.vector.tensor_tensor(t1, res[kt][:, sl], mub_bf,
                                                OP.subtract)
                        t2 = P.tile([128, 512], BF, tag="scr", bufs=5,
                                    name=f"t2_{nc.next_id()}")
                        nc.vector.scalar_tensor_tensor(
                            t2, t1, col(cs + kt), rsb_bf,
                            op0=OP.mult, op1=OP.mult)
                        nc.scalar.activation(out=dst[kt][:, sl], in_=t2,
                                             func=AF.Identity,
                                             bias=col(cb + kt))

            # ---------------- transformer layers ----------------
            for l in range(NLAYERS):
                AB = l * PL
                fw_sb = load_fw(l)       # lands during attention

                probe("xt", xT[0])
                qT, qT_tag = alloc_act()
                proj_v(qT, AW_Q, AB + 0, xT)
                probe("q", qT[0])
                kT, kT_tag = alloc_act()
                proj_v(kT, AW_K, AB + 4, xT)
                probe("k", kT[0])

                # V token-major (ones-row matmul adds bias)
                for rt in range(8):
                    ps = Q.tile([128, 512], F32, tag="C", bufs=2,
                                name=f"v_{nc.next_id()}")
                    for kt in range(DT):
                        MM(ps, xT[kt][:, rt * 128:(rt + 1) * 128],
                           aw_sb[:, AW_V + kt * 512:AW_V + kt * 512 + 512],
                           start=(kt == 0), stop=False)
                    MM(ps, ones_row_r, bvr_sb[l], start=False, stop=True)
                    v3o = vv[rt].rearrange("p (h e) -> p h e", e=DK + 1)
                    nc.vector.tensor_copy(
                        v3o[:, :, 0:DK],
                        ps.rearrange("p (h d) -> p h d", d=DK))

                probe("v", vv[0])
                gT, gT_tag = alloc_act()
                proj_v(gT, AW_G, AB + 12, xT, act=AF.Tanh)
                probe("g", gT[0])

                # ---------------- attention ----------------
                ctxT, ctx_tag = alloc_act()
                for b in range(BL):
                    for hp in range(4):
                        prt = [[None, None], [None, None]]
                        for h01 in range(2):
                            hs = slice(h01 * 64, h01 * 64 + 64)
                            for jp in range(2):
                                psA = Q.tile([128, 1024], F32, tag="A",
                                             bufs=2,
                                             name=f"sc_{nc.next_id()}")
                                for j2 in range(2):
                                    jt = jp * 2 + j2
                                    MM(psA[:, j2 * 512:(j2 + 1) * 512],
                                       kT[hp][hs, b * 512 + jt * 128:
                                              b * 512 + jt * 128 + 128],
                                       qT[hp][hs, b * 512:(b + 1) * 512],
                                       start=True, stop=True)
                                pr = P.tile([128, 1024], BF, tag="pr",
                                            bufs=4,
                                            name=f"pr_{nc.next_id()}")
                                nc.scalar.activation(out=pr, in_=psA,
                                                     func=AF.Exp)
                                nc.vector.tensor_mul(
                                    pr, pr,
                                    posE_sb[:, jp * 1024:(jp + 1) * 1024])
                                probe("pr", pr)
                                prt[h01][jp] = pr
                        for h01 in range(2):
                            h = hp * 2 + h01
                            pc = Q.tile([128, 512], F32, tag="B", bufs=2,
                                        name=f"pc_{nc.next_id()}")
                            for jt in range(4):
                                MM(pc[0:DK + 1, :],
                                   vv[b * 4 + jt][:, h * (DK + 1):
                                                  (h + 1) * (DK + 1)],
                                   prt[h01][jt // 2][:, (jt % 2) * 512:
                                                     (jt % 2) * 512 + 512],
                                   start=(jt == 0), stop=(jt == 3))
                            probe("pc", pc[0:DK + 1, :])
                            dcp = P.tile([1, 512], F32, tag="rden", bufs=3,
                                         name=f"dc_{nc.next_id()}")
                            nc.vector.tensor_copy(dcp, pc[DK:DK + 1, :])
                            rden = P.tile([1, 512], F32, tag="rden", bufs=3,
                                          name=f"rd_{nc.next_id()}")
                            nc.vector.reciprocal_approx_fast(
                                out=rden, in_=dcp)
                            rdr = P.tile([1, 512], F32R, tag="rden", bufs=3,
                                         name=f"rdr_{nc.next_id()}")
                            nc.vector.tensor_copy(rdr, rden)
                            pbc = Q.tile([64, 512], F32, tag="C", bufs=2,
                                         name=f"bc_{nc.next_id()}")
                            MM(pbc, ones_row_r[:, 0:64], rdr,
                               start=True, stop=True)
                            ctmp = P.tile([64, 512], BF, tag="ctmp", bufs=3,
                                          name=f"ct_{nc.next_id()}")
                            nc.scalar.activation(out=ctmp, in_=pc[0:DK, :],
                                                 func=AF.Copy)
                            nc.vector.tensor_mul(
                                ctxT[hp][h01 * 64:h01 * 64 + 64,
                                         b * 512:(b + 1) * 512],
                                ctmp, pbc)
                free_act(qT_tag)
                free_act(kT_tag)

                probe("ctx", ctxT[0])
                attT, attT_tag = alloc_act()
                proj_v(attT, AW_O, AB + 8, ctxT)
                probe("att", attT[0])
                free_act(ctx_tag)

                # res = x + (tanh+1) * att'  (att' pre-halved via Wo')
                res, res_tag = alloc_act()
                for kt in range(DT):
                    for rc in range(2):
                        sl = slice(rc * 512, (rc + 1) * 512)
                        tm = P.tile([128, 512], BF, tag="scr", bufs=5,
                                    name=f"tm_{nc.next_id()}")
                        nc.vector.scalar_tensor_tensor(
                            tm, gT[kt][:, sl], 1.0, attT[kt][:, sl],
                            op0=OP.add, op1=OP.mult)
                        nc.vector.tensor_add(res[kt][:, sl], tm,
                                             xT[kt][:, sl])
                free_act(xT_tag)
                free_act(gT_tag)
                free_act(attT_tag)

                probe("res", res[0])
                x1, x1_tag = alloc_act()
                layernorm(res, AB + 16, AB + 20, x1)
                probe("x1", x1[0])
                free_act(res_tag)

                # prefetch next layer's attention weights
                if l + 1 < NLAYERS:
                    aw_next = load_aw(l + 1)

                # ---------------- FF ----------------
                res2, res2_tag = alloc_act()
                for rc in range(2):
                    sl = slice(rc * 512, (rc + 1) * 512)
                    accA = [Q.tile([128, 1024], F32, tag="A", bufs=2,
                                   name=f"fa_{nc.next_id()}")
                            for _ in range(2)]
                    accs = [accA[0][:, 0:512], accA[0][:, 512:1024],
                            accA[1][:, 0:512], accA[1][:, 512:1024]]
                    for nt in range(FT):
                        pg = Q.tile([128, 512], F32, tag="C", bufs=2,
                                    name=f"pg_{nc.next_id()}")
                        for kt in range(DT):
                            MM(pg,
                               fw_sb[:, FW_G + kt * 2048 + nt * 128:
                                     FW_G + kt * 2048 + nt * 128 + 128],
                               x1[kt][:, sl],
                               start=(kt == 0), stop=(kt == DT - 1))
                        p1 = Q.tile([128, 512], F32, tag="B", bufs=2,
                                    name=f"p1_{nc.next_id()}")
                        for kt in range(DT):
                            MM(p1,
                               fw_sb[:, FW_1 + kt * 2048 + nt * 128:
                                     FW_1 + kt * 2048 + nt * 128 + 128],
                               x1[kt][:, sl],
                               start=(kt == 0), stop=(kt == DT - 1))
                        a1 = P.tile([128, 512], BF, tag="fsc", bufs=5,
                                    name=f"a1_{nc.next_id()}")
                        nc.scalar.activation(out=a1, in_=p1,
                                             func=AF.Identity,
                                             bias=col(AB + 32 + nt))
                        tg = P.tile([128, 512], BF, tag="fsc", bufs=5,
                                    name=f"tg_{nc.next_id()}")
                        nc.scalar.activation(out=tg, in_=pg, func=AF.Tanh,
                                             bias=col(AB + 48 + nt))
                        f = P.tile([128, 512], BF, tag="fsc", bufs=5,
                                   name=f"f_{nc.next_id()}")
                        nc.vector.scalar_tensor_tensor(
                            f, tg, 1.0, a1, op0=OP.add, op1=OP.mult)
                        for dt_ in range(DT):
                            MM(accs[dt_],
                               fw_sb[:, FW_2 + nt * 512 + dt_ * 128:
                                     FW_2 + nt * 512 + dt_ * 128 + 128],
                               f, start=(nt == 0), stop=(nt == FT - 1))
                    for dt_ in range(DT):
                        nc.vector.scalar_tensor_tensor(
                            res2[dt_][:, sl], accs[dt_], col(AB + 64 + dt_),
                            x1[dt_][:, sl], op0=OP.add, op1=OP.add)
                probe("res2", res2[0])
                free_act(x1_tag)

                xT, xT_tag = alloc_act()
                layernorm(res2, AB + 24, AB + 28, xT)
                probe("xout", xT[0])
                free_act(res2_tag)
                if l + 1 < NLAYERS:
                    aw_sb = aw_next

            # ---------------- head ----------------
            hT = []
            for kt in range(DT):
                xr = P.tile([128, BL], F32, tag="hd", bufs=8,
                            name=f"xr_{nc.next_id()}")
                nc.vector.tensor_reduce(
                    xr, xT[kt].rearrange("p (b s) -> p b s", b=BL),
                    axis=mybir.AxisListType.X, op=OP.add)
                ht = P.tile([128, BL], BF, tag="hT", bufs=8,
                            name=f"hm_{nc.next_id()}")
                nc.vector.tensor_scalar(out=ht, in0=xr, scalar1=1.0 / S,
                                        scalar2=None, op0=OP.mult)
                hT.append(ht)
            for nt in range(DT):
                ps = Q.tile([128, BL], F32, tag="B", bufs=2,
                            name=f"ho_{nc.next_id()}")
                MM(ps, ow_sb[:, nt * 128:(nt + 1) * 128], xo_sb,
                   start=True, stop=True)
                ht = P.tile([128, BL], BF, tag="hT", bufs=8,
                            name=f"hx_{nc.next_id()}")
                nc.vector.tensor_scalar(out=ht, in0=ps,
                                        scalar1=col(HB + 4 + nt),
                                        scalar2=None, op0=OP.add)
                hT.append(ht)

            eps1 = eps2[0:1, :]

            def head_ln_relu(zt, n_tiles, nfeat, cs, cb, outtag):
                s1p = Q.tile([1, BL], F32, tag="B", bufs=2,
                             name=f"hs1_{nc.next_id()}")
                for kt in range(n_tiles):
                    MM(s1p, ones_col, zt[kt], start=(kt == 0),
                       stop=(kt == n_tiles - 1))
                s2p = Q.tile([1, BL], F32, tag="C", bufs=2,
                             name=f"hs2_{nc.next_id()}")
                for kt in range(n_tiles):
                    z2 = P.tile([128, BL], BF, tag="hd2", bufs=4,
                                name=f"z2_{nc.next_id()}")
                    nc.vector.tensor_mul(z2, zt[kt], zt[kt])
                    MM(s2p, ones_col, z2, start=(kt == 0),
                       stop=(kt == n_tiles - 1))
                mu = P.tile([1, BL], F32R, tag="hmu", bufs=4,
                            name=f"hmu_{nc.next_id()}")
                nc.vector.tensor_scalar(out=mu, in0=s1p,
                                        scalar1=1.0 / nfeat,
                                        scalar2=None, op0=OP.mult)
                m2 = P.tile([1, BL], F32, tag="hln", bufs=8,
                            name=f"hm2_{nc.next_id()}")
                nc.vector.tensor_scalar(out=m2, in0=s2p,
                                        scalar1=1.0 / nfeat,
                                        scalar2=None, op0=OP.mult)
                var = P.tile([1, BL], F32, tag="hln", bufs=8,
                             name=f"hva_{nc.next_id()}")
                nc.vector.scalar_tensor_tensor(
                    var, mu, -1.0, mu, op0=OP.mult, op1=OP.mult)
                nc.vector.tensor_add(var, var, m2)
                sq = P.tile([1, BL], F32, tag="hln", bufs=8,
                            name=f"hsq_{nc.next_id()}")
                nc.scalar.activation(out=sq, in_=var, func=AF.Sqrt,
                                     bias=eps1)
                rs = P.tile([1, BL], F32, tag="hmu", bufs=4,
                            name=f"hrs_{nc.next_id()}")
                nc.vector.reciprocal_approx_fast(out=rs, in_=sq)
                rsr = P.tile([1, BL], F32R, tag="hmu", bufs=4,
                             name=f"hrr_{nc.next_id()}")
                nc.vector.tensor_copy(rsr, rs)
                mub = Q.tile([128, BL], F32, tag="C", bufs=2,
                             name=f"hmb_{nc.next_id()}")
                MM(mub, ones_row_r, mu, start=True, stop=True)
                rsb = Q.tile([128, BL], F32, tag="B", bufs=2,
                             name=f"hrb_{nc.next_id()}")
                MM(rsb, ones_row_r, rsr, start=True, stop=True)
                outs = []
                for kt in range(n_tiles):
                    t1 = P.tile([128, BL], F32, tag="hd", bufs=8,
                                name=f"ht1_{nc.next_id()}")
                    nc.vector.tensor_tensor(t1, zt[kt], mub, OP.subtract)
                    t2 = P.tile([128, BL], F32, tag="hd", bufs=8,
                                name=f"ht2_{nc.next_id()}")
                    nc.vector.scalar_tensor_tensor(
                        t2, t1, col(cs + kt), rsb, op0=OP.mult, op1=OP.mult)
                    o = P.tile([128, BL], BF, tag=outtag, bufs=4,
                               name=f"ho_{nc.next_id()}")
                    nc.scalar.activation(out=o, in_=t2, func=AF.Relu,
                                         bias=col(cb + kt))
                    outs.append(o)
                return outs

            # fc1 [1024 -> 256]
            z1 = []
            for nt in range(2):
                ps = Q.tile([128, BL], F32, tag="B", bufs=2,
                            name=f"f1_{nc.next_id()}")
                for kt in range(8):
                    MM(ps, hw1_sb[:, kt * 256 + nt * 128:
                                  kt * 256 + nt * 128 + 128], hT[kt],
                       start=(kt == 0), stop=(kt == 7))
                z = P.tile([128, BL], BF, tag="z1", bufs=2,
                           name=f"z1_{nc.next_id()}")
                nc.vector.tensor_scalar(out=z, in0=ps,
                                        scalar1=col(HB + 8 + nt),
                                        scalar2=None, op0=OP.add)
                z1.append(z)
            h1 = head_ln_relu(z1, 2, 256, HB + 10, HB + 12, "h1")

            # fc2 [256 -> 128]
            ps = Q.tile([128, BL], F32, tag="B", bufs=2,
                        name=f"f2_{nc.next_id()}")
            for kt in range(2):
                MM(ps, hw2_sb[:, kt * 128:(kt + 1) * 128], h1[kt],
                   start=(kt == 0), stop=(kt == 1))
            z2_ = P.tile([128, BL], BF, tag="z2", bufs=2,
                         name=f"z2h_{nc.next_id()}")
            nc.vector.tensor_scalar(out=z2_, in0=ps, scalar1=col(HB + 14),
                                    scalar2=None, op0=OP.add)
            h2 = head_ln_relu([z2_], 1, 128, HB + 15, HB + 16, "h2")

            # fc3 [128 -> 1]
            ps = Q.tile([1, BL], F32, tag="C", bufs=2,
                        name=f"f3_{nc.next_id()}")
            MM(ps, hw2_sb[:, 256:257], h2[0], start=True, stop=True)
            out_sb = P.tile([1, BL], F32, tag="outsb", bufs=1)
            nc.vector.tensor_scalar(out=out_sb, in0=ps,
                                    scalar1=pb_sb[0:1, HB + 18:HB + 19],
                                    scalar2=None, op0=OP.add)
            nc.sync.dma_start(out=out_ext[:, :], in_=out_sb)

    nc.compile()
    return nc


def _tile_w(W):
    """[K*128, Dout] -> [128, K*Dout] bf16 (kt-major blocks)."""
    K = W.shape[0] // 128
    return np.ascontiguousarray(
        W.reshape(K, 128, -1).transpose(1, 0, 2).reshape(128, -1)
    ).astype(NPBF)


def _cols(pb, base, vec):
    """Pack vec[len=128*n] into pb[:, base:base+n] column-major."""
    v = np.asarray(vec, np.float32).reshape(-1, 128).T
    pb[:, base:base + v.shape[1]] = v


def _pack_shared(inputs):
    f32 = np.float32
    g = lambda k: np.asarray(inputs[k], f32)

    aw = np.zeros((L, 128, AW_COLS), NPBF)
    fw = np.zeros((L, 128, FW_COLS), NPBF)
    pb = np.zeros((128, NP), f32)
    bvr = np.zeros((L, HDK), f32)
    Wq, bq = g("Wq"), g("bq")
    Wk, bk = g("Wk"), g("bk")
    Wv, bv = g("Wv"), g("bv")
    Wo, bo = g("Wo"), g("bo")
    Wg, bg = g("Wg"), g("bg")
    Wf1, bf1 = g("Wf1"), g("bf1")
    Wfg, bfg = g("Wfg"), g("bfg")
    Wf2, bf2 = g("Wf2"), g("bf2")
    for l in range(L):
        aw[l][:, AW_Q:AW_K] = _tile_w(Wq[l] * 0.125)
        aw[l][:, AW_K:AW_V] = _tile_w(Wk[l])
        aw[l][:, AW_V:AW_O] = _tile_w(Wv[l])
        aw[l][:, AW_O:AW_G] = _tile_w(Wo[l] * 0.5)
        aw[l][:, AW_G:] = _tile_w(Wg[l] * 0.5)
        fw[l][:, FW_1:FW_G] = _tile_w(Wf1[l] * 0.5)
        fw[l][:, FW_G:FW_2] = _tile_w(Wfg[l] * 0.5)
        fw[l][:, FW_2:] = _tile_w(Wf2[l])
        AB = l * PL
        _cols(pb, AB + 0, bq[l] * 0.125)
        _cols(pb, AB + 4, bk[l])
        _cols(pb, AB + 8, bo[l] * 0.5)
        _cols(pb, AB + 12, bg[l] * 0.5)
        _cols(pb, AB + 16, g("ln1_s")[l])
        _cols(pb, AB + 20, g("ln1_b")[l])
        _cols(pb, AB + 24, g("ln2_s")[l])
        _cols(pb, AB + 28, g("ln2_b")[l])
        _cols(pb, AB + 32, bf1[l] * 0.5)
        _cols(pb, AB + 48, bfg[l] * 0.5)
        _cols(pb, AB + 64, bf2[l])
        bvr[l] = bv[l]
    _cols(pb, HB + 0, g("cgm_b"))
    _cols(pb, HB + 4, g("other_b"))
    _cols(pb, HB + 8, g("fb1"))
    _cols(pb, HB + 10, g("fln1_s"))
    _cols(pb, HB + 12, g("fln1_b"))
    pb[:, HB + 14] = g("fb2")
    pb[:, HB + 15] = g("fln2_s")
    pb[:, HB + 16] = g("fln2_b")
    pb[:, HB + 17] = g("fW3")[:, 0]
    pb[0, HB + 18] = g("fb3")[0]

    # posE: exp(pos_bias) in scores-transposed layout
    rbar = g("rel_emb").mean(axis=1)            # [1023]
    posE = np.zeros((128, 2048), f32)
    Jv = np.arange(128)[:, None]
    Iv = np.arange(512)[None, :]
    for jt in range(4):
        idx = 511 - 128 * jt - Jv + Iv
        posE[:, jt * 512:(jt + 1) * 512] = np.exp(rbar[idx])

    return {
        "cgmW": g("cgm_W").astype(NPBF),
        "posE": posE.astype(NPBF),
        "aw": aw, "fw": fw, "pb": pb, "bvr": bvr,
        "hw1": _tile_w(g("fW1")),
        "hw2": np.concatenate([_tile_w(g("fW2")),
                               g("fW3").astype(NPBF)], axis=1),
        "ow": g("other_W").astype(NPBF),
    }


def _get_nc():
    if "nc" not in _CACHE:
        _CACHE["nc"] = _build()
    return _CACHE["nc"]


def kernel(**inputs):
    shared = _pack_shared(inputs)
    x_cgm = np.asarray(inputs["x_cgm"], np.float32)
    x_other = np.asarray(inputs["x_other"], np.float32)
    in_maps = []
    for c in range(NCORES):
        m = dict(shared)
        xs = x_cgm[c * BL:(c + 1) * BL].reshape(R, FC).T
        m["xin"] = np.ascontiguousarray(xs).astype(NPBF)
        m["xo"] = np.ascontiguousarray(
            x_other[c * BL:(c + 1) * BL].T).astype(NPBF)
        in_maps.append(m)

    nc = _get_nc()
    trace = bool(int(os.environ.get("KTRACE", "0")))
    res = run_bass_kernel_spmd(nc, in_maps, core_ids=list(range(NCORES)),
                               trace=trace)
    _CACHE["last_res"] = res
    out = np.concatenate(
        [res.results[c]["out"].reshape(BL, 1) for c in range(NCORES)], axis=0)
    return out.astype(np.float32)


# revision 17
# speedup vs baseline: 1.2011x; 1.2011x over previous
"""Trainium2 Bass kernel for nn_AttentionModel (4-layer gated transformer).

Sharding: pure data-parallel over batch (B=16 -> 2 per core, 8 cores, no
collectives). Feature-major activations in bf16 (fp32 PSUM accumulate).

Perf structure:
- Weights host-packed to bf16 blobs; one DMA per layer stage (attn 2.6MB,
  ff 6.3MB), prefetched a stage ahead. All biases/LN params in one fp32
  [128, NP] blob -> single DMA.
- All activation functions from the exp_and_others table set (exp/tanh/
  identity/copy/square) except LN Sqrt: sigmoid(z) folded to
  (tanh(z/2)+1) * 0.5 with the 0.5s folded into weights host-side.
- Softmax: probs = exp(s)*exp(p); exp(pos_bias) precomputed on host (bf16
  multiplicative factor) so scores go PSUM -ACT-> exp -TT-> probs (bf16).
  Score scale 1/8 folded into Wq. Denominators via ones-augmented V column;
  reciprocal_approx_fast (~5x faster than vector.reciprocal).
- LayerNorm: partition sums via PE ones-matmuls, batched stats for both
  512-token chunks in one [2,512] tile, recip_approx for 1/sigma.
"""

import os
import sys

for _p in ("/opt/trn_rl_repo",):
    if os.path.isdir(_p) and _p not in sys.path:
        sys.path.insert(0, _p)

import numpy as np
import ml_dtypes

import concourse.bass as bass
import concourse.mybir as mybir
import concourse.tile as tile
from concourse import bacc
from concourse.bass_utils import run_bass_kernel_spmd

F32 = mybir.dt.float32
F32R = mybir.dt.float32r
BF = mybir.dt.bfloat16
NPBF = ml_dtypes.bfloat16
AF = mybir.ActivationFunctionType
OP = mybir.AluOpType

B, S, FC, FO = 16, 512, 24, 16
D, H, DK, FFD, L = 512, 8, 64, 2048, 4
MAXPOS = 512
EPS = 1e-6

NCORES = 8
BL = B // NCORES          # local batch = 2
R = BL * S                # local tokens = 1024
DT = D // 128             # feature tiles = 4
FT = FFD // 128           # ff tiles = 16
HDK = H * DK

# aw blob column bases (per layer, [128, 10240] bf16)
AW_Q, AW_K, AW_V, AW_O, AW_G = 0, 2048, 4096, 6144, 8192
AW_COLS = 10240
# fw blob column bases ([128, 24576] bf16)
FW_1, FW_G, FW_2 = 0, 8192, 16384
FW_COLS = 24576
# param blob columns (fp32 [128, NP])
PL = 68                   # per-layer stride
# per-layer: bq 0, bk 4, bo 8, bg 12, l1s 16, l1b 20, l2s 24, l2b 28,
#            bf1 32, bfg 48, bf2 64
HB = L * PL               # head base = 272
# head: cgm_b +0, other_b +4, fb1 +8, fl1s +10, fl1b +12, fb2 +14,
#       fl2s +15, fl2b +16, fw3 +17, fb3 +18 (row 0)
NP = HB + 19

_CACHE = {}


def _build():
    nc = bacc.Bacc("TRN2", target_bir_lowering=False, debug=False,
                   num_devices=NCORES)

    def par(name, shape, dt):
        return nc.declare_dram_parameter(name, list(shape), dt, isOutput=False)

    xin_d = par("xin", [FC, R], BF)
    xo_d = par("xo", [FO, BL], BF)
    cgmW_d = par("cgmW", [FC, D], BF)
    posE_d = par("posE", [128, 4 * 512], BF)
    aw_d = par("aw", [L, 128, AW_COLS], BF)
    fw_d = par("fw", [L, 128, FW_COLS], BF)
    pb_d = par("pb", [128, NP], F32)
    bvr_d = par("bvr", [L, HDK], F32R)
    hw1_d = par("hw1", [128, 8 * 256], BF)
    hw2_d = par("hw2", [128, 2 * 128 + 1], BF)
    ow_d = par("ow", [FO, D], BF)
    out_ext = nc.declare_dram_parameter("out", [1, BL], F32, isOutput=True)

    with tile.TileContext(nc) as tc:
        with (
            nc.allow_low_precision(reason="bf16 matmul/activation pipeline"),
            tc.tile_pool(name="P", bufs=1) as P,
            tc.tile_pool(name="Q", bufs=1, space="PSUM") as Q,
        ):
            MM = nc.tensor.matmul
            NLAYERS = int(os.environ.get("KLAYERS", L))
            KPROBE = os.environ.get("KPROBE", "")
            if KPROBE:
                dbg_ext = nc.declare_dram_parameter(
                    "dbg", [128, 1024], F32, isOutput=True)
                dbg_done = [False]

                def probe(name, ap):
                    if name != KPROBE or dbg_done[0]:
                        return
                    dbg_done[0] = True
                    pdim = ap.shape[0]
                    fdim = ap.free_size()
                    dt_ = P.tile([128, 1024], F32, tag="dbgt", bufs=1)
                    nc.vector.memset(dt_, 0.0)
                    nc.vector.tensor_copy(
                        dt_[0:pdim, 0:fdim], ap)
                    nc.sync.dma_start(out=dbg_ext[:, :], in_=dt_)
            else:
                def probe(name, ap):
                    pass

            # ---------------- constants ----------------
            ones_col = P.tile([128, 1], BF, tag="c_oc", bufs=1)
            nc.vector.memset(ones_col, 1.0)
            ones_row_f = P.tile([1, 128], F32, tag="c_orf", bufs=1)
            nc.vector.memset(ones_row_f, 1.0)
            ones_row_r = P.tile([1, 128], F32R, tag="c_orr", bufs=1)
            nc.vector.tensor_copy(ones_row_r, ones_row_f)
            eps2 = P.tile([2, 1], F32, tag="c_e", bufs=1)
            nc.vector.memset(eps2, EPS)

            # ---------------- persistent loads ----------------
            pb_sb = P.tile([128, NP], F32, tag="pb", bufs=1)
            nc.sync.dma_start(out=pb_sb, in_=pb_d[:, :])
            posE_sb = P.tile([128, 2048], BF, tag="posE", bufs=1)
            nc.sync.dma_start(out=posE_sb, in_=posE_d[:, :])
            xin_sb = P.tile([FC, R], BF, tag="xin", bufs=1)
            nc.sync.dma_start(out=xin_sb, in_=xin_d[:, :])
            cgmW_sb = P.tile([FC, D], BF, tag="cgmW", bufs=1)
            nc.sync.dma_start(out=cgmW_sb, in_=cgmW_d[:, :])
            bvr_sb = []
            for l in range(L):
                t = P.tile([1, HDK], F32R, tag="bvr", bufs=L,
                           name=f"bvr{l}")
                nc.sync.dma_start(out=t, in_=bvr_d[l].unsqueeze(0))
                bvr_sb.append(t)

            def col(c, n=1):
                return pb_sb[:, c:c + n]

            # layer weight pools
            def load_aw(l, chunked=False):
                t = P.tile([128, AW_COLS], BF, tag="aw",
                           bufs=(1 if KPROBE else 2),
                           name=f"aw{l}")
                if chunked:
                    for c0 in range(0, AW_COLS, 2048):
                        nc.sync.dma_start(out=t[:, c0:c0 + 2048],
                                          in_=aw_d[l][:, c0:c0 + 2048])
                else:
                    nc.sync.dma_start(out=t, in_=aw_d[l])
                return t

            def load_fw(l):
                t = P.tile([128, FW_COLS], BF, tag="fw", bufs=1,
                           name=f"fw{l}")
                nc.sync.dma_start(out=t, in_=fw_d[l])
                return t

            aw_sb = load_aw(0, chunked=True)
            xo_sb = P.tile([FO, BL], BF, tag="xo", bufs=1)
            nc.sync.dma_start(out=xo_sb, in_=xo_d[:, :])
            ow_sb = P.tile([FO, D], BF, tag="ow", bufs=1)
            nc.sync.dma_start(out=ow_sb, in_=ow_d[:, :])
            hw1_sb = P.tile([128, 2048], BF, tag="hw1", bufs=1)
            nc.sync.dma_start(out=hw1_sb, in_=hw1_d[:, :])
            hw2_sb = P.tile([128, 257], BF, tag="hw2", bufs=1)
            nc.sync.dma_start(out=hw2_sb, in_=hw2_d[:, :])

            # ------------- activation tile allocator -------------
            free_tags = ["bA", "bB", "bC", "bD", "bE", "bF"]

            def alloc_act():
                tag = free_tags.pop(0)
                tiles = [P.tile([128, R], BF, tag=tag, bufs=4,
                                name=f"{tag}_{nc.next_id()}")
                         for _ in range(DT)]
                return tiles, tag

            def free_act(tag):
                free_tags.append(tag)

            # persistent token-major V (ones-augmented)
            vv = []
            for rt in range(8):
                t = P.tile([128, H * (DK + 1)], BF, tag="vv", bufs=8,
                           name=f"vv{rt}")
                v3 = t.rearrange("p (h e) -> p h e", e=DK + 1)
                nc.vector.memset(v3[:, :, DK:DK + 1], 1.0)
                vv.append(t)

            # ---------------- input projection ----------------
            xT, xT_tag = alloc_act()
            for nt in range(DT):
                for rc in range(2):
                    ps = Q.tile([128, 512], F32, tag="B", bufs=2)
                    MM(ps, cgmW_sb[:, nt * 128:(nt + 1) * 128],
                       xin_sb[:, rc * 512:(rc + 1) * 512],
                       start=True, stop=True)
                    nc.scalar.activation(
                        out=xT[nt][:, rc * 512:(rc + 1) * 512], in_=ps,
                        func=AF.Identity, bias=col(HB + nt))

            # ---------------- helpers ----------------
            def proj_v(dst, wbase, bcols, src, act=None):
                """dst[nt] = act(src @ W + b), feature-major.

                act=None: vector tensor_scalar add-bias (PSUM->bf16)
                act=AF.*: scalar activation with bias
                """
                for nt in range(DT):
                    for rc in range(2):
                        ps = Q.tile([128, 512], F32, tag="B", bufs=2,
                                    name=f"pj_{nc.next_id()}")
                        for kt in range(DT):
                            MM(ps,
                               aw_sb[:, wbase + kt * 512 + nt * 128:
                                     wbase + kt * 512 + nt * 128 + 128],
                               src[kt][:, rc * 512:(rc + 1) * 512],
                               start=(kt == 0), stop=(kt == DT - 1))
                        o = dst[nt][:, rc * 512:(rc + 1) * 512]
                        if act is None:
                            nc.vector.tensor_scalar(
                                out=o, in0=ps, scalar1=col(bcols + nt),
                                scalar2=None, op0=OP.add)
                        else:
                            nc.scalar.activation(out=o, in_=ps, func=act,
                                                 bias=col(bcols + nt))

            def layernorm(res, cs, cb, dst):
                """dst = LN(res) over features (partitions)."""
                for rc in range(2):
                    sl = slice(rc * 512, (rc + 1) * 512)
                    s1p = Q.tile([1, 512], F32, tag="B", bufs=2,
                                 name=f"s1_{nc.next_id()}")
                    s2p = Q.tile([1, 512], F32, tag="C", bufs=2,
                                 name=f"s2_{nc.next_id()}")
                    for kt in range(DT):
                        MM(s1p, ones_col, res[kt][:, sl],
                           start=(kt == 0), stop=(kt == DT - 1))
                    for kt in range(DT):
                        sq = P.tile([128, 512], BF, tag="scr", bufs=5,
                                    name=f"sq_{nc.next_id()}")
                        nc.vector.tensor_mul(sq, res[kt][:, sl],
                                             res[kt][:, sl])
                        MM(s2p, ones_col, sq,
                           start=(kt == 0), stop=(kt == DT - 1))
                    mu = P.tile([1, 512], F32R, tag="ln_mu", bufs=3,
                                name=f"mu_{nc.next_id()}")
                    nc.vector.tensor_scalar(out=mu, in0=s1p,
                                            scalar1=1.0 / D,
                                            scalar2=None, op0=OP.mult)
                    m2 = P.tile([1, 512], F32, tag="ln_t", bufs=3,
                                name=f"m2_{nc.next_id()}")
                    nc.vector.tensor_scalar(out=m2, in0=s2p,
                                            scalar1=1.0 / D,
                                            scalar2=None, op0=OP.mult)
                    var = P.tile([1, 512], F32, tag="ln_t", bufs=3,
                                 name=f"va_{nc.next_id()}")
                    nc.vector.scalar_tensor_tensor(
                        var, mu, -1.0, mu, op0=OP.mult, op1=OP.mult)
                    nc.vector.tensor_add(var, var, m2)
                    sg = P.tile([1, 512], F32, tag="ln_t", bufs=3,
                                name=f"sg_{nc.next_id()}")
                    nc.scalar.activation(out=sg, in_=var, func=AF.Sqrt,
                                         bias=eps2[0:1, :])
                    rs = P.tile([1, 512], F32, tag="ln_mu", bufs=3,
                                name=f"rs_{nc.next_id()}")
                    nc.vector.reciprocal_approx_fast(out=rs, in_=sg)
                    rsr = P.tile([1, 512], F32R, tag="ln_mu", bufs=3,
                                 name=f"rsr_{nc.next_id()}")
                    nc.vector.tensor_copy(rsr, rs)
                    mub = Q.tile([128, 512], F32, tag="C", bufs=2,
                                 name=f"mb_{nc.next_id()}")
                    MM(mub, ones_row_r, mu, start=True, stop=True)
                    rsb = Q.tile([128, 512], F32, tag="B", bufs=2,
                                 name=f"rb_{nc.next_id()}")
                    MM(rsb, ones_row_r, rsr, start=True, stop=True)
                    mub_bf = P.tile([128, 512], BF, tag="scr", bufs=5,
                                    name=f"mbb_{nc.next_id()}")
                    nc.scalar.activation(out=mub_bf, in_=mub, func=AF.Copy)
                    rsb_bf = P.tile([128, 512], BF, tag="scr", bufs=5,
                                    name=f"rbb_{nc.next_id()}")
                    nc.scalar.activation(out=rsb_bf, in_=rsb, func=AF.Copy)
                    for kt in range(DT):
                        t1 = P.tile([128, 512], BF, tag="scr", bufs=5,
                                    name=f"t1_{nc.next_id()}")
                        nc.vector.tensor_tensor(t1, res[kt][:, sl], mub_bf,
                                                OP.subtract)
                        t2 = P.tile([128, 512], BF, tag="scr", bufs=5,
                                    name=f"t2_{nc.next_id()}")
                        nc.vector.scalar_tensor_tensor(
                            t2, t1, col(cs + kt), rsb_bf,
                            op0=OP.mult, op1=OP.mult)
                        nc.scalar.activation(out=dst[kt][:, sl], in_=t2,
                                             func=AF.Identity,
                                             bias=col(cb + kt))

            # ---------------- transformer layers ----------------
            for l in range(NLAYERS):
                AB = l * PL
                fw_sb = load_fw(l)       # lands during attention

                probe("xt", xT[0])
                qT, qT_tag = alloc_act()
                proj_v(qT, AW_Q, AB + 0, xT)
                probe("q", qT[0])
                kT, kT_tag = alloc_act()
                proj_v(kT, AW_K, AB + 4, xT)
                probe("k", kT[0])

                # V token-major (ones-row matmul adds bias)
                for rt in range(8):
                    ps = Q.tile([128, 512], F32, tag="C", bufs=2,
                                name=f"v_{nc.next_id()}")
                    for kt in range(DT):
                        MM(ps, xT[kt][:, rt * 128:(rt + 1) * 128],
                           aw_sb[:, AW_V + kt * 512:AW_V + kt * 512 + 512],
                           start=(kt == 0), stop=False)
                    MM(ps, ones_row_r, bvr_sb[l], start=False, stop=True)
                    v3o = vv[rt].rearrange("p (h e) -> p h e", e=DK + 1)
                    nc.vector.tensor_copy(
                        v3o[:, :, 0:DK],
                        ps.rearrange("p (h d) -> p h d", d=DK))

                probe("v", vv[0])
                gT, gT_tag = alloc_act()
                proj_v(gT, AW_G, AB + 12, xT, act=AF.Tanh)
                probe("g", gT[0])

                # ---------------- attention ----------------
                ctxT, ctx_tag = alloc_act()
                for b in range(BL):
                    for hp in range(4):
                        prt = [[None, None], [None, None]]
                        for h01 in range(2):
                            hs = slice(h01 * 64, h01 * 64 + 64)
                            for jp in range(2):
                                psA = Q.tile([128, 1024], F32, tag="A",
                                             bufs=2,
                                             name=f"sc_{nc.next_id()}")
                                for j2 in range(2):
                                    jt = jp * 2 + j2
                                    MM(psA[:, j2 * 512:(j2 + 1) * 512],
                                       kT[hp][hs, b * 512 + jt * 128:
                                              b * 512 + jt * 128 + 128],
                                       qT[hp][hs, b * 512:(b + 1) * 512],
                                       start=True, stop=True)
                                pr = P.tile([128, 1024], BF, tag="pr",
                                            bufs=4,
                                            name=f"pr_{nc.next_id()}")
                                nc.scalar.activation(out=pr, in_=psA,
                                                     func=AF.Exp)
                                nc.vector.tensor_mul(
                                    pr, pr,
                                    posE_sb[:, jp * 1024:(jp + 1) * 1024])
                                probe("pr", pr)
                                prt[h01][jp] = pr
                        for h01 in range(2):
                            h = hp * 2 + h01
                            pc = Q.tile([128, 512], F32, tag="B", bufs=2,
                                        name=f"pc_{nc.next_id()}")
                            for jt in range(4):
                                MM(pc[0:DK + 1, :],
                                   vv[b * 4 + jt][:, h * (DK + 1):
                                                  (h + 1) * (DK + 1)],
                                   prt[h01][jt // 2][:, (jt % 2) * 512:
                                                     (jt % 2) * 512 + 512],
                                   start=(jt == 0), stop=(jt == 3))
                            probe("pc", pc[0:DK + 1, :])
                            dcp = P.tile([1, 512], F32, tag="rden", bufs=3,
                                         name=f"dc_{nc.next_id()}")
                            nc.vector.tensor_copy(dcp, pc[DK:DK + 1, :])
                            rden = P.tile([1, 512], F32, tag="rden", bufs=3,
                                          name=f"rd_{nc.next_id()}")
                            nc.vector.reciprocal_approx_fast(
                                out=rden, in_=dcp)
                            rdr = P.tile([1, 512], F32R, tag="rden", bufs=3,
                                         name=f"rdr_{nc.next_id()}")
                            nc.vector.tensor_copy(rdr, rden)
                            pbc = Q.tile([64, 512], F32, tag="C", bufs=2,
                                         name=f"bc_{nc.next_id()}")
                            MM(pbc, ones_row_r[:, 0:64], rdr,
                               start=True, stop=True)
                            ctmp = P.tile([64, 512], BF, tag="ctmp", bufs=3,
                                          name=f"ct_{nc.next_id()}")
                            nc.scalar.activation(out=ctmp, in_=pc[0:DK, :],
                                                 func=AF.Copy)
                            nc.vector.tensor_mul(
                                ctxT[hp][h01 * 64:h01 * 64 + 64,
                                         b * 512:(b + 1) * 512],
                                ctmp, pbc)
                free_act(qT_tag)
                free_act(kT_tag)

                probe("ctx", ctxT[0])
                attT, attT_tag = alloc_act()
                proj_v(attT, AW_O, AB + 8, ctxT)
                probe("att", attT[0])
                free_act(ctx_tag)

                # res = x + (tanh+1) * att'  (att' pre-halved via Wo')
                res, res_tag = alloc_act()
                for kt in range(DT):
                    for rc in range(2):
                        sl = slice(rc * 512, (rc + 1) * 512)
                        tm = P.tile([128, 512], BF, tag="scr", bufs=5,
                                    name=f"tm_{nc.next_id()}")
                        nc.vector.scalar_tensor_tensor(
                            tm, gT[kt][:, sl], 1.0, attT[kt][:, sl],
                            op0=OP.add, op1=OP.mult)
                        nc.vector.tensor_add(res[kt][:, sl], tm,
                                             xT[kt][:, sl])
                free_act(xT_tag)
                free_act(gT_tag)
                free_act(attT_tag)

                probe("res", res[0])
                x1, x1_tag = alloc_act()
                layernorm(res, AB + 16, AB + 20, x1)
                probe("x1", x1[0])
                free_act(res_tag)

                # prefetch next layer's attention weights
                if l + 1 < NLAYERS:
                    aw_next = load_aw(l + 1)

                # ---------------- FF ----------------
                res2, res2_tag = alloc_act()
                for rc in range(2):
                    sl = slice(rc * 512, (rc + 1) * 512)
                    accA = [Q.tile([128, 1024], F32, tag="A", bufs=2,
                                   name=f"fa_{nc.next_id()}")
                            for _ in range(2)]
                    accs = [accA[0][:, 0:512], accA[0][:, 512:1024],
                            accA[1][:, 0:512], accA[1][:, 512:1024]]
                    for nt in range(FT):
                        pg = Q.tile([128, 512], F32, tag="C", bufs=2,
                                    name=f"pg_{nc.next_id()}")
                        for kt in range(DT):
                            MM(pg,
                               fw_sb[:, FW_G + kt * 2048 + nt * 128:
                                     FW_G + kt * 2048 + nt * 128 + 128],
                               x1[kt][:, sl],
                               start=(kt == 0), stop=(kt == DT - 1))
                        p1 = Q.tile([128, 512], F32, tag="B", bufs=2,
                                    name=f"p1_{nc.next_id()}")
                        for kt in range(DT):
                            MM(p1,
                               fw_sb[:, FW_1 + kt * 2048 + nt * 128:
                                     FW_1 + kt * 2048 + nt * 128 + 128],
                               x1[kt][:, sl],
                               start=(kt == 0), stop=(kt == DT - 1))
                        a1 = P.tile([128, 512], BF, tag="fsc", bufs=5,
                                    name=f"a1_{nc.next_id()}")
                        nc.vector.tensor_scalar(
                            out=a1, in0=p1, scalar1=col(AB + 32 + nt),
                            scalar2=None, op0=OP.add)
                        tg = P.tile([128, 512], BF, tag="fsc", bufs=5,
                                    name=f"tg_{nc.next_id()}")
                        nc.scalar.activation(out=tg, in_=pg, func=AF.Tanh,
                                             bias=col(AB + 48 + nt))
                        f = P.tile([128, 512], BF, tag="fsc", bufs=5,
                                   name=f"f_{nc.next_id()}")
                        nc.vector.scalar_tensor_tensor(
                            f, tg, 1.0, a1, op0=OP.add, op1=OP.mult)
                        for dt_ in range(DT):
                            MM(accs[dt_],
                               fw_sb[:, FW_2 + nt * 512 + dt_ * 128:
                                     FW_2 + nt * 512 + dt_ * 128 + 128],
                               f, start=(nt == 0), stop=(nt == FT - 1))
                    for dt_ in range(DT):
                        nc.vector.scalar_tensor_tensor(
                            res2[dt_][:, sl], accs[dt_], col(AB + 64 + dt_),
                            x1[dt_][:, sl], op0=OP.add, op1=OP.add)
                probe("res2", res2[0])
                free_act(x1_tag)

                xT, xT_tag = alloc_act()
                layernorm(res2, AB + 24, AB + 28, xT)
                probe("xout", xT[0])
                free_act(res2_tag)
                if l + 1 < NLAYERS:
                    aw_sb = aw_next

            # ---------------- head ----------------
            hT = []
            for kt in range(DT):
                xr = P.tile([128, BL], F32, tag="hd", bufs=8,
                            name=f"xr_{nc.next_id()}")
                nc.vector.tensor_reduce(
                    xr, xT[kt].rearrange("p (b s) -> p b s", b=BL),
                    axis=mybir.AxisListType.X, op=OP.add)
                ht = P.tile([128, BL], BF, tag="hT", bufs=8,
                            name=f"hm_{nc.next_id()}")
                nc.vector.tensor_scalar(out=ht, in0=xr, scalar1=1.0 / S,
                                        scalar2=None, op0=OP.mult)
                hT.append(ht)
            for nt in range(DT):
                ps = Q.tile([128, BL], F32, tag="B", bufs=2,
                            name=f"ho_{nc.next_id()}")
                MM(ps, ow_sb[:, nt * 128:(nt + 1) * 128], xo_sb,
                   start=True, stop=True)
                ht = P.tile([128, BL], BF, tag="hT", bufs=8,
                            name=f"hx_{nc.next_id()}")
                nc.vector.tensor_scalar(out=ht, in0=ps,
                                        scalar1=col(HB + 4 + nt),
                                        scalar2=None, op0=OP.add)
                hT.append(ht)

            eps1 = eps2[0:1, :]

            def head_ln_relu(zt, n_tiles, nfeat, cs, cb, outtag):
                s1p = Q.tile([1, BL], F32, tag="B", bufs=2,
                             name=f"hs1_{nc.next_id()}")
                for kt in range(n_tiles):
                    MM(s1p, ones_col, zt[kt], start=(kt == 0),
                       stop=(kt == n_tiles - 1))
                s2p = Q.tile([1, BL], F32, tag="C", bufs=2,
                             name=f"hs2_{nc.next_id()}")
                for kt in range(n_tiles):
                    z2 = P.tile([128, BL], BF, tag="hd2", bufs=4,
                                name=f"z2_{nc.next_id()}")
                    nc.vector.tensor_mul(z2, zt[kt], zt[kt])
                    MM(s2p, ones_col, z2, start=(kt == 0),
                       stop=(kt == n_tiles - 1))
                mu = P.tile([1, BL], F32R, tag="hmu", bufs=4,
                            name=f"hmu_{nc.next_id()}")
                nc.vector.tensor_scalar(out=mu, in0=s1p,
                                        scalar1=1.0 / nfeat,
                                        scalar2=None, op0=OP.mult)
                m2 = P.tile([1, BL], F32, tag="hln", bufs=8,
                            name=f"hm2_{nc.next_id()}")
                nc.vector.tensor_scalar(out=m2, in0=s2p,
                                        scalar1=1.0 / nfeat,
                                        scalar2=None, op0=OP.mult)
                var = P.tile([1, BL], F32, tag="hln", bufs=8,
                             name=f"hva_{nc.next_id()}")
                nc.vector.scalar_tensor_tensor(
                    var, mu, -1.0, mu, op0=OP.mult, op1=OP.mult)
                nc.vector.tensor_add(var, var, m2)
                sq = P.tile([1, BL], F32, tag="hln", bufs=8,
                            name=f"hsq_{nc.next_id()}")
                nc.scalar.activation(out=sq, in_=var, func=AF.Sqrt,
                                     bias=eps1)
                rs = P.tile([1, BL], F32, tag="hmu", bufs=4,
                            name=f"hrs_{nc.next_id()}")
                nc.vector.reciprocal_approx_fast(out=rs, in_=sq)
                rsr = P.tile([1, BL], F32R, tag="hmu", bufs=4,
                             name=f"hrr_{nc.next_id()}")
                nc.vector.tensor_copy(rsr, rs)
                mub = Q.tile([128, BL], F32, tag="C", bufs=2,
                             name=f"hmb_{nc.next_id()}")
                MM(mub, ones_row_r, mu, start=True, stop=True)
                rsb = Q.tile([128, BL], F32, tag="B", bufs=2,
                             name=f"hrb_{nc.next_id()}")
                MM(rsb, ones_row_r, rsr, start=True, stop=True)
                outs = []
                for kt in range(n_tiles):
                    t1 = P.tile([128, BL], F32, tag="hd", bufs=8,
                                name=f"ht1_{nc.next_id()}")
                    nc.vector.tensor_tensor(t1, zt[kt], mub, OP.subtract)
                    t2 = P.tile([128, BL], F32, tag="hd", bufs=8,
                                name=f"ht2_{nc.next_id()}")
                    nc.vector.scalar_tensor_tensor(
                        t2, t1, col(cs + kt), rsb, op0=OP.mult, op1=OP.mult)
                    o = P.tile([128, BL], BF, tag=outtag, bufs=4,
                               name=f"ho_{nc.next_id()}")
                    nc.scalar.activation(out=o, in_=t2, func=AF.Relu,
                                         bias=col(cb + kt))
                    outs.append(o)
                return outs

            # fc1 [1024 -> 256]
            z1 = []
            for nt in range(2):
                ps = Q.tile([128, BL], F32, tag="B", bufs=2,
                            name=f"f1_{nc.next_id()}")
                for kt in range(8):
                    MM(ps, hw1_sb[:, kt * 256 + nt * 128:
                                  kt * 256 + nt * 128 + 128], hT[kt],
                       start=(kt == 0), stop=(kt == 7))
                z = P.tile([128, BL], BF, tag="z1", bufs=2,
                           name=f"z1_{nc.next_id()}")
                nc.vector.tensor_scalar(out=z, in0=ps,
                                        scalar1=col(HB + 8 + nt),
                                        scalar2=None, op0=OP.add)
                z1.append(z)
            h1 = head_ln_relu(z1, 2, 256, HB + 10, HB + 12, "h1")

            # fc2 [256 -> 128]
            ps = Q.tile([128, BL], F32, tag="B", bufs=2,
                        name=f"f2_{nc.next_id()}")
            for kt in range(2):
                MM(ps, hw2_sb[:, kt * 128:(kt + 1) * 128], h1[kt],
                   start=(kt == 0), stop=(kt == 1))
            z2_ = P.tile([128, BL], BF, tag="z2", bufs=2,
                         name=f"z2h_{nc.next_id()}")
            nc.vector.tensor_scalar(out=z2_, in0=ps, scalar1=col(HB + 14),
                                    scalar2=None, op0=OP.add)
            h2 = head_ln_relu([z2_], 1, 128, HB + 15, HB + 16, "h2")

            # fc3 [128 -> 1]
            ps = Q.tile([1, BL], F32, tag="C", bufs=2,
                        name=f"f3_{nc.next_id()}")
            MM(ps, hw2_sb[:, 256:257], h2[0], start=True, stop=True)
            out_sb = P.tile([1, BL], F32, tag="outsb", bufs=1)
            nc.vector.tensor_scalar(out=out_sb, in0=ps,
                                    scalar1=pb_sb[0:1, HB + 18:HB + 19],
                                    scalar2=None, op0=OP.add)
            nc.sync.dma_start(out=out_ext[:, :], in_=out_sb)

    nc.compile()
    return nc


def _tile_w(W):
    """[K*128, Dout] -> [128, K*Dout] bf16 (kt-major blocks)."""
    K = W.shape[0] // 128
    return np.ascontiguousarray(
        W.reshape(K, 128, -1).transpose(1, 0, 2).reshape(128, -1)
    ).astype(NPBF)


def _cols(pb, base, vec):
    """Pack vec[len=128*n] into pb[:, base:base+n] column-major."""
    v = np.asarray(vec, np.float32).reshape(-1, 128).T
    pb[:, base:base + v.shape[1]] = v


def _pack_shared(inputs):
    f32 = np.float32
    g = lambda k: np.asarray(inputs[k], f32)

    aw = np.zeros((L, 128, AW_COLS), NPBF)
    fw = np.zeros((L, 128, FW_COLS), NPBF)
    pb = np.zeros((128, NP), f32)
    bvr = np.zeros((L, HDK), f32)
    Wq, bq = g("Wq"), g("bq")
    Wk, bk = g("Wk"), g("bk")
    Wv, bv = g("Wv"), g("bv")
    Wo, bo = g("Wo"), g("bo")
    Wg, bg = g("Wg"), g("bg")
    Wf1, bf1 = g("Wf1"), g("bf1")
    Wfg, bfg = g("Wfg"), g("bfg")
    Wf2, bf2 = g("Wf2"), g("bf2")
    for l in range(L):
        aw[l][:, AW_Q:AW_K] = _tile_w(Wq[l] * 0.125)
        aw[l][:, AW_K:AW_V] = _tile_w(Wk[l])
        aw[l][:, AW_V:AW_O] = _tile_w(Wv[l])
        aw[l][:, AW_O:AW_G] = _tile_w(Wo[l] * 0.5)
        aw[l][:, AW_G:] = _tile_w(Wg[l] * 0.5)
        fw[l][:, FW_1:FW_G] = _tile_w(Wf1[l] * 0.5)
        fw[l][:, FW_G:FW_2] = _tile_w(Wfg[l] * 0.5)
        fw[l][:, FW_2:] = _tile_w(Wf2[l])
        AB = l * PL
        _cols(pb, AB + 0, bq[l] * 0.125)
        _cols(pb, AB + 4, bk[l])
        _cols(pb, AB + 8, bo[l] * 0.5)
        _cols(pb, AB + 12, bg[l] * 0.5)
        _cols(pb, AB + 16, g("ln1_s")[l])
        _cols(pb, AB + 20, g("ln1_b")[l])
        _cols(pb, AB + 24, g("ln2_s")[l])
        _cols(pb, AB + 28, g("ln2_b")[l])
        _cols(pb, AB + 32, bf1[l] * 0.5)
        _cols(pb, AB + 48, bfg[l] * 0.5)
        _cols(pb, AB + 64, bf2[l])
        bvr[l] = bv[l]
    _cols(pb, HB + 0, g("cgm_b"))
    _cols(pb, HB + 4, g("other_b"))
    _cols(pb, HB + 8, g("fb1"))
    _cols(pb, HB + 10, g("fln1_s"))
    _cols(pb, HB + 12, g("fln1_b"))
    pb[:, HB + 14] = g("fb2")
    pb[:, HB + 15] = g("fln2_s")
    pb[:, HB + 16] = g("fln2_b")
    pb[:, HB + 17] = g("fW3")[:, 0]
    pb[0, HB + 18] = g("fb3")[0]

    # posE: exp(pos_bias) in scores-transposed layout
    rbar = g("rel_emb").mean(axis=1)            # [1023]
    posE = np.zeros((128, 2048), f32)
    Jv = np.arange(128)[:, None]
    Iv = np.arange(512)[None, :]
    for jt in range(4):
        idx = 511 - 128 * jt - Jv + Iv
        posE[:, jt * 512:(jt + 1) * 512] = np.exp(rbar[idx])

    return {
        "cgmW": g("cgm_W").astype(NPBF),
        "posE": posE.astype(NPBF),
        "aw": aw, "fw": fw, "pb": pb, "bvr": bvr,
        "hw1": _tile_w(g("fW1")),
        "hw2": np.concatenate([_tile_w(g("fW2")),
                               g("fW3").astype(NPBF)], axis=1),
        "ow": g("other_W").astype(NPBF),
    }


def _get_nc():
    if "nc" not in _CACHE:
        _CACHE["nc"] = _build()
    return _CACHE["nc"]


def kernel(**inputs):
    shared = _pack_shared(inputs)
    x_cgm = np.asarray(inputs["x_cgm"], np.float32)
    x_other = np.asarray(inputs["x_other"], np.float32)
    in_maps = []
    for c in range(NCORES):
        m = dict(shared)
        xs = x_cgm[c * BL:(c + 1) * BL].reshape(R, FC).T
        m["xin"] = np.ascontiguousarray(xs).astype(NPBF)
        m["xo"] = np.ascontiguousarray(
            x_other[c * BL:(c + 1) * BL].T).astype(NPBF)
        in_maps.append(m)

    nc = _get_nc()
    trace = bool(int(os.environ.get("KTRACE", "0")))
    res = run_bass_kernel_spmd(nc, in_maps, core_ids=list(range(NCORES)),
                               trace=trace)
    _CACHE["last_res"] = res
    out = np.concatenate(
        [res.results[c]["out"].reshape(BL, 1) for c in range(NCORES)], axis=0)
    return out.astype(np.float32)
